# revision 22
# baseline (speedup 1.0000x reference)
"""Trainium2 Bass kernel for nn_AsymmResBlock (submanifold sparse conv block).

    shortcut = act_bn(conv(feats, nbr133, W00), bn00)
    shortcut = act_bn(conv(shortcut, nbr313, W01), bn01)
    res      = act_bn(conv(feats, nbr313, W10), bn10)
    res      = act_bn(conv(res, nbr133, W11), bn11)
    out      = res + shortcut
with conv(f, nbr, W)[n] = sum_k f[nbr[k, n]] @ W[k]  (9 offsets, -1 = none).

The voxel grid is ~2.7% dense: for k != 4 only ~2.7% of entries are valid,
and k == 4 is the identity. Each conv = dense matmul with W[4] over the
core's own shard (kept feature-major, no gather) + a compacted sparse
correction: valid (k, n, j) pairs, sorted by j and bucketed into static
32768-row windows, are bulk-gathered (dma_gather, int16 window-relative
indices), multiplied by W[k] on the PE, and scatter-added (dma_scatter_add,
int16 local rows) into a per-conv correction table. The dense pass folds the
correction into the PSUM accumulator with transpose-accumulate matmuls and
applies LeakyReLU+BN as s*x + b + relu(-0.99*s*x) on the Scalar engine.
Intermediates cross cores via one AllGather per branch. The program is
JIT-specialized to the actual tables (shared SPMD skeleton; per-core index
payloads are inputs).
"""
import numpy as np

import concourse.bass as bass
import concourse.bacc as bacc
import concourse.mybir as mybir
import concourse.tile as tile
from concourse.bass_utils import run_bass_kernel_spmd
from concourse.library_config import mlp as _mlp_lib
from concourse.masks import make_identity

N = 200000
CIN, COUT = 64, 128
NCORES = 8
P = 128
LEAK = 0.01
EPS = 1e-5

SHARD = N // NCORES          # 25000
SLOT = 25088                 # padded shard (196 tiles of 128)
GRP = 512
NG = SLOT // GRP             # 49 dense groups
AGR = NCORES * SLOT          # 200704 rows in allgathered tables
TRASH = SLOT                 # corr rows [SLOT, SLOT+128) catch padding
WIN = 32768                  # static gather window size (int16 range)
KS = [0, 1, 2, 3, 5, 6, 7, 8]


def _wrap16(vals):
    """int16 values (len % 16 == 0) -> dma_gather/scatter index layout
    [128, len/16]: entry n at [n % 16, n // 16], replicated to 128 rows."""
    n = len(vals)
    w = np.zeros((16, n // 16), np.int16)
    w[np.arange(n) % 16, np.arange(n) // 16] = vals.astype(np.int16)
    return np.tile(w, (8, 1))


def _bn_vecs(bn):
    gamma, beta, mean, var = [np.asarray(a, np.float64) for a in np.asarray(bn)]
    s = gamma / np.sqrt(var + EPS)
    b = beta - mean * s
    s2 = -(1.0 - LEAK) * s
    return s.astype(np.float32), s2.astype(np.float32), b.astype(np.float32)


def _sparse_plan(nbr, to_pos, table_rows):
    """Plan one conv's sparse correction.

    to_pos: callable (core, j_array) -> table rows. Returns (skel, gidx, sidx):
    skel = list of (k, [(win_base, tk), ...]) with tk = max over cores of
    ceil(count/128) 128-row batches for that window; gidx/sidx = per-core
    wrapped int16 index tensors [128, total/16]."""
    nwin = -(-table_rows // WIN)
    # bucket per core/k/window
    ent = {}
    for c in range(NCORES):
        for ki, k in enumerate(KS):
            seg = nbr[k, c * SHARD:(c + 1) * SHARD]
            ii = np.nonzero(seg >= 0)[0]
            jj = to_pos(c, seg[ii].astype(np.int64))
            w = jj // WIN
            for wi in range(nwin):
                m = w == wi
                ent[(c, ki, wi)] = (ii[m], jj[m] - wi * WIN)
    skel = []
    for ki, k in enumerate(KS):
        wins = []
        for wi in range(nwin):
            mx = max(len(ent[(c, ki, wi)][0]) for c in range(NCORES))
            tk = -(-mx // 128)
            if tk > 0:
                wins.append((wi * WIN, tk))
        skel.append((k, wins))
    gidx, sidx = [], []
    for c in range(NCORES):
        gparts, sparts = [], []
        for ki, k in enumerate(KS):
            for wi in range(nwin):
                base_tk = [t for (b, t) in skel[ki][1] if b == wi * WIN]
                if not base_tk:
                    continue
                tk = base_tk[0]
                ii, rel = ent[(c, ki, wi)]
                mp = tk * 128
                r = np.zeros(mp, np.int64)
                s = np.full(mp, TRASH, np.int64)
                r[:len(rel)] = rel
                s[:len(ii)] = ii
                gparts.append(_wrap16(r))
                sparts.append(_wrap16(s))
        if not gparts:
            gparts = [np.zeros((128, 8), np.int16)]
            sparts = [np.zeros((128, 8), np.int16)]
        gidx.append(np.concatenate(gparts, axis=1))
        sidx.append(np.concatenate(sparts, axis=1))
    return skel, np.stack(gidx), np.stack(sidx)


def build_program(skels, timing=False):
    """skels: dict conv-tag -> skeleton from _sparse_plan. timing=True builds
    a single-core collective-free twin for cost-model timing."""
    nc = bacc.Bacc("TRN2", debug=False, num_devices=(1 if timing else NCORES),
                   num_swdge_queues=NQ_G)
    f32, i16 = mybir.dt.float32, mybir.dt.int16

    feats_row = nc.dram_tensor("feats_row", [FSROWS[0], CIN], f32, kind="ExternalInput")
    ftf = nc.dram_tensor("ftf", [CIN, SLOT], f32, kind="ExternalInput")
    w00 = nc.dram_tensor("w00", [CIN, 9 * 128], f32, kind="ExternalInput")
    w10 = nc.dram_tensor("w10", [CIN, 9 * 128], f32, kind="ExternalInput")
    w01 = nc.dram_tensor("w01", [COUT, 9 * 128], f32, kind="ExternalInput")
    w11 = nc.dram_tensor("w11", [COUT, 9 * 128], f32, kind="ExternalInput")
    bnv = nc.dram_tensor("bnv", [P, 12], f32, kind="ExternalInput")
    gidx_t, sidx_t = {}, {}
    for tag in ("00", "10", "01", "11"):
        gcols = max(8, sum(t for (_, wins) in skels[tag] for (_, t) in wins) * 8)
        gidx_t[tag] = nc.dram_tensor(f"gidx{tag}", [P, gcols], i16, kind="ExternalInput")
        sidx_t[tag] = nc.dram_tensor(f"sidx{tag}", [P, gcols], i16, kind="ExternalInput")
    out_d = nc.dram_tensor("out", [SLOT, COUT], f32, kind="ExternalOutput")

    corr = {tag: nc.dram_tensor(f"corr{tag}", [SLOT + P, COUT], f32)
            for tag in ("00", "10", "01", "11")}
    sc0_row = nc.dram_tensor("sc0_row", [SLOT, COUT], f32)
    res0_row = nc.dram_tensor("res0_row", [SLOT, COUT], f32)
    ag_sc0 = nc.dram_tensor("ag_sc0", [AGR, COUT], f32, addr_space="Shared")
    ag_res0 = nc.dram_tensor("ag_res0", [AGR, COUT], f32, addr_space="Shared")
    ft_sc0 = nc.dram_tensor("ft_sc0", [COUT, SLOT], f32)
    ft_res0 = nc.dram_tensor("ft_res0", [COUT, SLOT], f32)
    sc_d = nc.dram_tensor("sc_d", [SLOT, COUT], f32)

    wsb_t = {"00": w00, "10": w10, "01": w01, "11": w11}
    cin_t = {"00": CIN, "10": CIN, "01": COUT, "11": COUT}
    src_t = {"00": feats_row, "10": feats_row, "01": ag_sc0, "11": ag_res0}
    ft_t = {"00": ftf, "10": ftf, "01": ft_sc0, "11": ft_res0}

    with tile.TileContext(nc) as tc:
        with (
            tc.tile_pool(name="const", bufs=1) as cpool,
            tc.tile_pool(name="sg", bufs=2) as gpool,
            tc.tile_pool(name="work", bufs=3) as epool,
            tc.tile_pool(name="pta_ps", bufs=2, space="PSUM") as ptpool,
            tc.tile_pool(name="acc_ps", bufs=2, space="PSUM") as accpool,
            tc.tile_pool(name="ot_ps", bufs=2, space="PSUM") as otpool,
        ):
            nc.gpsimd.load_library(_mlp_lib)

            ident = cpool.tile([P, P], f32)
            make_identity(nc, ident[:])

            w_sb = {}
            for tag in ("00", "10", "01", "11"):
                w_sb[tag] = cpool.tile([cin_t[tag], 9 * 128], f32, name=f"w{tag}_sb")
                nc.sync.dma_start(out=w_sb[tag][:, :], in_=wsb_t[tag][:, :])
            bn_sb = cpool.tile([P, 12], f32)
            nc.sync.dma_start(out=bn_sb[:], in_=bnv[:, :])

            zrow = cpool.tile([P, GRP], f32)
            nc.vector.memset(zrow[:, :], 0.0)
            for tag in ("00", "10", "01", "11"):
                for r in range(0, SLOT + P, GRP):
                    m = min(GRP, SLOT + P - r)
                    nc.sync.dma_start(out=corr[tag][r:r + m, :],
                                      in_=zrow[:, :m])

            def sparse_phase(tag):
                cin = cin_t[tag]
                src = src_t[tag]
                skel = skels[tag]
                gi = epool.tile([P, gidx_t[tag].shape[1]], i16, name=f"gi{tag}", tag="gi")
                nc.sync.dma_start(out=gi[:, :], in_=gidx_t[tag][:, :])
                si = epool.tile([P, sidx_t[tag].shape[1]], i16, name=f"si{tag}", tag="si")
                nc.sync.dma_start(out=si[:, :], in_=sidx_t[tag][:, :])
                col = 0
                for ki, (k, wins) in enumerate(skel):
                    tks = sum(t for (_, t) in wins)
                    if tks == 0:
                        continue
                    g = gpool.tile([P, tks, cin], f32, name=f"sg{tag}_{k}", tag="sg")
                    c0 = 0
                    for (wbase, tk) in wins:
                        nc.gpsimd.dma_gather(
                            g[:, c0:c0 + tk, :], src[wbase:, :],
                            gi[:, (col + c0) * 8:(col + c0 + tk) * 8],
                            tk * 128, tk * 128, cin,
                            queue_num=(ki % NQ_G),
                        )
                        c0 += tk
                    sc_sb = epool.tile([P, tks * 128], f32, name=f"sc{tag}_{k}", tag="scs")
                    for b0 in range(0, tks, 4):
                        nb = min(4, tks - b0)
                        gw = nb * 128
                        pt = ptpool.tile([P, gw], f32, name=f"spt{tag}_{k}_{b0}",
                                         tag="pta", space="PSUM")
                        for j in range(nb):
                            nc.tensor.transpose(
                                out=pt[0:cin, j * 128:(j + 1) * 128],
                                in_=g[:, b0 + j, :], identity=ident[:])
                        gt = epool.tile([P, gw], f32, name=f"sgt{tag}_{k}_{b0}", tag="gt")
                        nc.vector.tensor_copy(out=gt[0:cin, :], in_=pt[0:cin, :])
                        sacc = accpool.tile([P, gw], f32, name=f"sacc{tag}_{k}_{b0}",
                                            tag="acc", space="PSUM")
                        nc.tensor.matmul(
                            out=sacc[:, :],
                            lhsT=w_sb[tag][:, k * 128:(k + 1) * 128],
                            rhs=gt[0:cin, :], start=True, stop=True)
                        sy = epool.tile([P, gw], f32, name=f"sy{tag}_{k}_{b0}", tag="sy")
                        nc.vector.tensor_copy(out=sy[:, :], in_=sacc[:, :])
                        sot = otpool.tile([P, gw], f32, name=f"sot{tag}_{k}_{b0}",
                                          tag="ot", space="PSUM")
                        for j in range(nb):
                            nc.tensor.transpose(
                                out=sot[:, j * 128:(j + 1) * 128],
                                in_=sy[:, j * 128:(j + 1) * 128], identity=ident[:])
                        nc.scalar.copy(out=sc_sb[:, b0 * 128:b0 * 128 + gw], in_=sot[:, :])
                    nc.gpsimd.dma_scatter_add(
                        corr[tag][:, :],
                        sc_sb[:, :].rearrange("p (b c) -> p b c", c=128),
                        si[:, col * 8:(col + tks) * 8],
                        tks * 128, tks * 128, COUT,
                        queue_num=(ki % NQ_G),
                    )
                    col += tks

            def dense_phase(tag):
                cin = cin_t[tag]
                ft = ft_t[tag]
                cidx = {"00": 0, "01": 3, "10": 6, "11": 9}[tag]
                s_ap = bn_sb[:, cidx:cidx + 1]
                s2_ap = bn_sb[:, cidx + 1:cidx + 2]
                b_ap = bn_sb[:, cidx + 2:cidx + 3]
                for gidx in range(NG):
                    c0 = gidx * GRP
                    rhs = epool.tile([cin, GRP], f32, name=f"rhs{tag}_{gidx}", tag="rhs")
                    nc.sync.dma_start(out=rhs[:, :], in_=ft[:, c0:c0 + GRP])
                    crr = epool.tile([P, 4, 128], f32, name=f"crr{tag}_{gidx}", tag="crr")
                    nc.sync.dma_start(
                        out=crr[:, :, :],
                        in_=corr[tag][c0:c0 + GRP, :].rearrange("(j p) c -> p j c", p=128))
                    acc = accpool.tile([P, GRP], f32, name=f"dacc{tag}_{gidx}",
                                       tag="acc", space="PSUM")
                    nc.tensor.matmul(out=acc[:, :],
                                     lhsT=w_sb[tag][:, 4 * 128:5 * 128],
                                     rhs=rhs[:, :], start=True, stop=False)
                    for j in range(4):
                        nc.tensor.matmul(out=acc[:, j * 128:(j + 1) * 128],
                                         lhsT=crr[:, j, :], rhs=ident[:],
                                         is_transpose=True, start=False, stop=(j == 3),
                                         skip_group_check=True)
                    t1 = epool.tile([P, GRP], f32, name=f"t1{tag}_{gidx}", tag="t1")
                    nc.scalar.activation(t1[:, :], acc[:, :],
                                         mybir.ActivationFunctionType.Identity,
                                         bias=b_ap, scale=s_ap)
                    r2 = epool.tile([P, GRP], f32, name=f"r2{tag}_{gidx}", tag="r2")
                    nc.scalar.activation(r2[:, :], acc[:, :],
                                         mybir.ActivationFunctionType.Relu,
                                         bias=0.0, scale=s2_ap)
                    y = epool.tile([P, GRP], f32, name=f"y{tag}_{gidx}", tag="y")
                    nc.vector.tensor_add(out=y[:, :], in0=t1[:, :], in1=r2[:, :])
                    if tag == "00":
                        nc.sync.dma_start(out=ft_sc0[:, c0:c0 + GRP], in_=y[:, :])
                    elif tag == "10":
                        nc.sync.dma_start(out=ft_res0[:, c0:c0 + GRP], in_=y[:, :])
                    ot = otpool.tile([P, GRP], f32, name=f"dot{tag}_{gidx}",
                                     tag="ot", space="PSUM")
                    for j in range(4):
                        nc.tensor.transpose(out=ot[:, j * 128:(j + 1) * 128],
                                            in_=y[:, j * 128:(j + 1) * 128],
                                            identity=ident[:])
                    yr = epool.tile([P, GRP], f32, name=f"yr{tag}_{gidx}", tag="yr")
                    nc.vector.tensor_copy(out=yr[:, :], in_=ot[:, :])
                    rview = "p (j c) -> p j c"
                    if tag == "00":
                        nc.sync.dma_start(
                            out=sc0_row[c0:c0 + GRP, :].rearrange("(j p) c -> p j c", p=128),
                            in_=yr[:, :].rearrange(rview, c=128))
                    elif tag == "10":
                        nc.sync.dma_start(
                            out=res0_row[c0:c0 + GRP, :].rearrange("(j p) c -> p j c", p=128),
                            in_=yr[:, :].rearrange(rview, c=128))
                    elif tag == "01":
                        nc.sync.dma_start(
                            out=sc_d[c0:c0 + GRP, :].rearrange("(j p) c -> p j c", p=128),
                            in_=yr[:, :].rearrange(rview, c=128))
                    else:
                        sct = epool.tile([P, GRP], f32, name=f"sct{tag}_{gidx}", tag="sct")
                        nc.sync.dma_start(
                            out=sct[:, :].rearrange(rview, c=128),
                            in_=sc_d[c0:c0 + GRP, :].rearrange("(j p) c -> p j c", p=128))
                        yf = epool.tile([P, GRP], f32, name=f"yf{tag}_{gidx}", tag="yf")
                        nc.vector.tensor_add(out=yf[:, :], in0=yr[:, :], in1=sct[:, :])
                        nc.sync.dma_start(
                            out=out_d[c0:c0 + GRP, :].rearrange("(j p) c -> p j c", p=128),
                            in_=yf[:, :].rearrange(rview, c=128))

            sparse_phase("00")
            sparse_phase("10")
            dense_phase("00")
            if not timing:
                nc.gpsimd.collective_compute(
                    "AllGather", mybir.AluOpType.bypass,
                    replica_groups=[list(range(NCORES))],
                    ins=[sc0_row[:, :]], outs=[ag_sc0[:, :]])
            dense_phase("10")
            if not timing:
                nc.gpsimd.collective_compute(
                    "AllGather", mybir.AluOpType.bypass,
                    replica_groups=[list(range(NCORES))],
                    ins=[res0_row[:, :]], outs=[ag_res0[:, :]])
            sparse_phase("01")
            dense_phase("01")
            sparse_phase("11")
            dense_phase("11")

    nc.compile()
    return nc


NQ_G = 2
FSROWS = [0]  # static compacted sparse-source row count (set before build)

_prog_cache = {}
_last_skels = [None]  # for test-side timing builds


def _skel_key(skels):
    return tuple((tag, tuple((k, tuple(w)) for (k, w) in skels[tag]))
                 for tag in ("00", "10", "01", "11"))


def kernel(feats, W00, W01, W10, W11, bn00, bn01, bn10, bn11, nbr133, nbr313):
    feats = np.ascontiguousarray(np.asarray(feats, np.float32))
    nbr133 = np.asarray(nbr133, np.int32)
    nbr313 = np.asarray(nbr313, np.int32)

    # The kernel folds the k=4 (0,0,0) offset into a dense identity matmul.
    ar = np.arange(N, dtype=np.int64)
    if not (np.array_equal(nbr133[4], ar) and np.array_equal(nbr313[4], ar)):
        raise NotImplementedError("k=4 is not the identity offset")

    # Per-core compacted gather source for the feats-based convs: only the
    # rows actually referenced by the core's valid non-center entries.
    rows_c = []
    for c in range(NCORES):
        js = []
        for nbr in (nbr133, nbr313):
            seg = nbr[KS, c * SHARD:(c + 1) * SHARD]
            js.append(seg[seg >= 0].astype(np.int64))
        rows_c.append(np.unique(np.concatenate(js)))
    fsmax = max(1, max(len(r) for r in rows_c))
    fsmax = -(-fsmax // 128) * 128
    FSROWS[0] = fsmax

    feat_pos = lambda c, j: np.searchsorted(rows_c[c], j)
    shard_pos = lambda c, j: (j // SHARD) * SLOT + j % SHARD

    skel00, g00, s00 = _sparse_plan(nbr133, feat_pos, fsmax)
    skel10, g10, s10 = _sparse_plan(nbr313, feat_pos, fsmax)
    skel01, g01, s01 = _sparse_plan(nbr313, shard_pos, AGR)
    skel11, s11g, s11s = _sparse_plan(nbr133, shard_pos, AGR)
    skels = {"00": skel00, "10": skel10, "01": skel01, "11": skel11}
    g11, s11 = s11g, s11s
    _last_skels[0] = skels

    key = (_skel_key(skels), fsmax)
    nc = _prog_cache.get(key)
    if nc is None:
        nc = build_program(skels)
        _prog_cache.clear()
        _prog_cache[key] = nc

    # weights: [9, ci, co] -> [ci, 9*co]
    wpack = lambda W: np.ascontiguousarray(
        np.asarray(W, np.float32).transpose(1, 0, 2).reshape(np.asarray(W).shape[1], -1))
    w00p, w10p = wpack(W00), wpack(W10)
    w01p, w11p = wpack(W01), wpack(W11)
    bnv = np.zeros((P, 12), np.float32)
    for i, bn in enumerate((bn00, bn01, bn10, bn11)):
        s, s2, b = _bn_vecs(bn)
        col = {0: 0, 1: 3, 2: 6, 3: 9}[i]
        bnv[:, col], bnv[:, col + 1], bnv[:, col + 2] = s, s2, b

    in_maps = []
    for c in range(NCORES):
        ftf = np.zeros((CIN, SLOT), np.float32)
        ftf[:, :SHARD] = feats[c * SHARD:(c + 1) * SHARD].T
        fsrc = np.zeros((fsmax, CIN), np.float32)
        fsrc[:len(rows_c[c])] = feats[rows_c[c]]
        in_maps.append({
            "feats_row": fsrc,
            "ftf": ftf,
            "w00": w00p, "w10": w10p, "w01": w01p, "w11": w11p,
            "bnv": bnv,
            "gidx00": g00[c], "sidx00": s00[c],
            "gidx10": g10[c], "sidx10": s10[c],
            "gidx01": g01[c], "sidx01": s01[c],
            "gidx11": g11[c], "sidx11": s11[c],
        })

    res = run_bass_kernel_spmd(nc, in_maps, core_ids=list(range(NCORES)))
    out = np.empty((N, COUT), np.float32)
    for c in range(NCORES):
        out[c * SHARD:(c + 1) * SHARD] = res.results[c]["out"][:SHARD]
    return out


# revision 24
# speedup vs baseline: 1.0085x; 1.0085x over previous
"""Trainium2 Bass kernel for nn_AsymmResBlock (submanifold sparse conv block).

    shortcut = act_bn(conv(feats, nbr133, W00), bn00)
    shortcut = act_bn(conv(shortcut, nbr313, W01), bn01)
    res      = act_bn(conv(feats, nbr313, W10), bn10)
    res      = act_bn(conv(res, nbr133, W11), bn11)
    out      = res + shortcut
with conv(f, nbr, W)[n] = sum_k f[nbr[k, n]] @ W[k]  (9 offsets, -1 = none).

The voxel grid is ~2.7% dense: for k != 4 only ~2.7% of entries are valid,
and k == 4 is the identity. Each conv = dense matmul with W[4] over the
core's own shard (kept feature-major, no gather) + a compacted sparse
correction: valid (k, n, j) pairs, sorted by j and bucketed into static
32768-row windows, are bulk-gathered (dma_gather, int16 window-relative
indices), multiplied by W[k] on the PE, and scatter-added (dma_scatter_add,
int16 local rows) into a per-conv correction table. The dense pass folds the
correction into the PSUM accumulator with transpose-accumulate matmuls and
applies LeakyReLU+BN as s*x + b + relu(-0.99*s*x) on the Scalar engine.
Intermediates cross cores via one AllGather per branch. The program is
JIT-specialized to the actual tables (shared SPMD skeleton; per-core index
payloads are inputs).
"""
import numpy as np

import concourse.bass as bass
import concourse.bacc as bacc
import concourse.mybir as mybir
import concourse.tile as tile
from concourse.bass_utils import run_bass_kernel_spmd
from concourse.library_config import mlp as _mlp_lib
from concourse.masks import make_identity

N = 200000
CIN, COUT = 64, 128
NCORES = 8
P = 128
LEAK = 0.01
EPS = 1e-5

SHARD = N // NCORES          # 25000
SLOT = 25088                 # padded shard (196 tiles of 128)
GRP = 512
NG = SLOT // GRP             # 49 dense groups
AGR = NCORES * SLOT          # 200704 rows in allgathered tables
TRASH = SLOT                 # corr rows [SLOT, SLOT+128) catch padding
WIN = 32768                  # static gather window size (int16 range)
KS = [0, 1, 2, 3, 5, 6, 7, 8]


def _wrap16(vals):
    """int16 values (len % 16 == 0) -> dma_gather/scatter index layout
    [128, len/16]: entry n at [n % 16, n // 16], replicated to 128 rows."""
    n = len(vals)
    w = np.zeros((16, n // 16), np.int16)
    w[np.arange(n) % 16, np.arange(n) // 16] = vals.astype(np.int16)
    return np.tile(w, (8, 1))


def _bn_vecs(bn):
    gamma, beta, mean, var = [np.asarray(a, np.float64) for a in np.asarray(bn)]
    s = gamma / np.sqrt(var + EPS)
    b = beta - mean * s
    s2 = -(1.0 - LEAK) * s
    return s.astype(np.float32), s2.astype(np.float32), b.astype(np.float32)


def _sparse_plan(nbr, to_pos, table_rows):
    """Plan one conv's sparse correction.

    to_pos: callable (core, j_array) -> table rows. Returns (skel, gidx, sidx):
    skel = list of (k, [(win_base, tk), ...]) with tk = max over cores of
    ceil(count/128) 128-row batches for that window; gidx/sidx = per-core
    wrapped int16 index tensors [128, total/16]."""
    nwin = -(-table_rows // WIN)
    # bucket per core/k/window
    ent = {}
    for c in range(NCORES):
        for ki, k in enumerate(KS):
            seg = nbr[k, c * SHARD:(c + 1) * SHARD]
            ii = np.nonzero(seg >= 0)[0]
            jj = to_pos(c, seg[ii].astype(np.int64))
            w = jj // WIN
            for wi in range(nwin):
                m = w == wi
                ent[(c, ki, wi)] = (ii[m], jj[m] - wi * WIN)
    skel = []
    for ki, k in enumerate(KS):
        wins = []
        for wi in range(nwin):
            mx = max(len(ent[(c, ki, wi)][0]) for c in range(NCORES))
            tk = -(-mx // 128)
            if tk > 0:
                wins.append((wi * WIN, tk))
        skel.append((k, wins))
    gidx, sidx = [], []
    for c in range(NCORES):
        gparts, sparts = [], []
        for ki, k in enumerate(KS):
            for wi in range(nwin):
                base_tk = [t for (b, t) in skel[ki][1] if b == wi * WIN]
                if not base_tk:
                    continue
                tk = base_tk[0]
                ii, rel = ent[(c, ki, wi)]
                mp = tk * 128
                r = np.zeros(mp, np.int64)
                s = np.full(mp, TRASH, np.int64)
                r[:len(rel)] = rel
                s[:len(ii)] = ii
                gparts.append(_wrap16(r))
                sparts.append(_wrap16(s))
        if not gparts:
            gparts = [np.zeros((128, 8), np.int16)]
            sparts = [np.zeros((128, 8), np.int16)]
        gidx.append(np.concatenate(gparts, axis=1))
        sidx.append(np.concatenate(sparts, axis=1))
    return skel, np.stack(gidx), np.stack(sidx)


def build_program(skels, timing=False):
    """skels: dict conv-tag -> skeleton from _sparse_plan. timing=True builds
    a single-core collective-free twin for cost-model timing."""
    nc = bacc.Bacc("TRN2", debug=False, num_devices=(1 if timing else NCORES),
                   num_swdge_queues=NQ_G)
    f32, i16 = mybir.dt.float32, mybir.dt.int16

    feats_row = nc.dram_tensor("feats_row", [FSROWS[0], CIN], f32, kind="ExternalInput")
    ftf = nc.dram_tensor("ftf", [CIN, SLOT], f32, kind="ExternalInput")
    w00 = nc.dram_tensor("w00", [CIN, 9 * 128], f32, kind="ExternalInput")
    w10 = nc.dram_tensor("w10", [CIN, 9 * 128], f32, kind="ExternalInput")
    w01 = nc.dram_tensor("w01", [COUT, 9 * 128], f32, kind="ExternalInput")
    w11 = nc.dram_tensor("w11", [COUT, 9 * 128], f32, kind="ExternalInput")
    bnv = nc.dram_tensor("bnv", [P, 12], f32, kind="ExternalInput")
    gidx_t, sidx_t = {}, {}
    for tag in ("00", "10", "01", "11"):
        gcols = max(8, sum(t for (_, wins) in skels[tag] for (_, t) in wins) * 8)
        gidx_t[tag] = nc.dram_tensor(f"gidx{tag}", [P, gcols], i16, kind="ExternalInput")
        sidx_t[tag] = nc.dram_tensor(f"sidx{tag}", [P, gcols], i16, kind="ExternalInput")
    out_d = nc.dram_tensor("out", [SLOT, COUT], f32, kind="ExternalOutput")

    bf16 = mybir.dt.bfloat16
    corr = {tag: nc.dram_tensor(f"corr{tag}", [SLOT + P, COUT], bf16)
            for tag in ("00", "10", "01", "11")}
    sc0_row = nc.dram_tensor("sc0_row", [SLOT, COUT], f32)
    res0_row = nc.dram_tensor("res0_row", [SLOT, COUT], f32)
    ag_sc0 = nc.dram_tensor("ag_sc0", [AGR, COUT], f32, addr_space="Shared")
    ag_res0 = nc.dram_tensor("ag_res0", [AGR, COUT], f32, addr_space="Shared")
    ft_sc0 = nc.dram_tensor("ft_sc0", [COUT, SLOT], f32)
    ft_res0 = nc.dram_tensor("ft_res0", [COUT, SLOT], f32)
    sc_d = nc.dram_tensor("sc_d", [SLOT, COUT], f32)

    wsb_t = {"00": w00, "10": w10, "01": w01, "11": w11}
    cin_t = {"00": CIN, "10": CIN, "01": COUT, "11": COUT}
    src_t = {"00": feats_row, "10": feats_row, "01": ag_sc0, "11": ag_res0}
    ft_t = {"00": ftf, "10": ftf, "01": ft_sc0, "11": ft_res0}

    with tile.TileContext(nc) as tc:
        with (
            tc.tile_pool(name="const", bufs=1) as cpool,
            tc.tile_pool(name="sg", bufs=2) as gpool,
            tc.tile_pool(name="work", bufs=3) as epool,
            tc.tile_pool(name="pta_ps", bufs=2, space="PSUM") as ptpool,
            tc.tile_pool(name="acc_ps", bufs=2, space="PSUM") as accpool,
            tc.tile_pool(name="ot_ps", bufs=2, space="PSUM") as otpool,
        ):
            nc.gpsimd.load_library(_mlp_lib)

            ident = cpool.tile([P, P], f32)
            make_identity(nc, ident[:])
            ident_bf = cpool.tile([P, P], mybir.dt.bfloat16)
            nc.vector.tensor_copy(out=ident_bf[:, :], in_=ident[:, :])

            w_sb = {}
            for tag in ("00", "10", "01", "11"):
                w_sb[tag] = cpool.tile([cin_t[tag], 9 * 128], f32, name=f"w{tag}_sb")
                nc.sync.dma_start(out=w_sb[tag][:, :], in_=wsb_t[tag][:, :])
            bn_sb = cpool.tile([P, 12], f32)
            nc.sync.dma_start(out=bn_sb[:], in_=bnv[:, :])

            zrow = cpool.tile([P, GRP], mybir.dt.bfloat16)
            nc.vector.memset(zrow[:, :], 0.0)
            for tag in ("00", "10", "01", "11"):
                for r in range(0, SLOT + P, GRP):
                    m = min(GRP, SLOT + P - r)
                    nc.sync.dma_start(out=corr[tag][r:r + m, :],
                                      in_=zrow[:, :m])

            def sparse_phase(tag):
                cin = cin_t[tag]
                src = src_t[tag]
                skel = skels[tag]
                gi = epool.tile([P, gidx_t[tag].shape[1]], i16, name=f"gi{tag}", tag="gi")
                nc.sync.dma_start(out=gi[:, :], in_=gidx_t[tag][:, :])
                si = epool.tile([P, sidx_t[tag].shape[1]], i16, name=f"si{tag}", tag="si")
                nc.sync.dma_start(out=si[:, :], in_=sidx_t[tag][:, :])
                col = 0
                for ki, (k, wins) in enumerate(skel):
                    tks = sum(t for (_, t) in wins)
                    if tks == 0:
                        continue
                    g = gpool.tile([P, tks, cin], f32, name=f"sg{tag}_{k}", tag="sg")
                    c0 = 0
                    for (wbase, tk) in wins:
                        nc.gpsimd.dma_gather(
                            g[:, c0:c0 + tk, :], src[wbase:, :],
                            gi[:, (col + c0) * 8:(col + c0 + tk) * 8],
                            tk * 128, tk * 128, cin,
                            queue_num=(ki % NQ_G),
                        )
                        c0 += tk
                    sc_sb = epool.tile([P, tks * 128], mybir.dt.bfloat16,
                                       name=f"sc{tag}_{k}", tag="scs")
                    for b0 in range(0, tks, 4):
                        nb = min(4, tks - b0)
                        gw = nb * 128
                        pt = ptpool.tile([P, gw], f32, name=f"spt{tag}_{k}_{b0}",
                                         tag="pta", space="PSUM")
                        for j in range(nb):
                            nc.tensor.transpose(
                                out=pt[0:cin, j * 128:(j + 1) * 128],
                                in_=g[:, b0 + j, :], identity=ident[:])
                        gt = epool.tile([P, gw], f32, name=f"sgt{tag}_{k}_{b0}", tag="gt")
                        nc.vector.tensor_copy(out=gt[0:cin, :], in_=pt[0:cin, :])
                        sacc = accpool.tile([P, gw], f32, name=f"sacc{tag}_{k}_{b0}",
                                            tag="acc", space="PSUM")
                        nc.tensor.matmul(
                            out=sacc[:, :],
                            lhsT=w_sb[tag][:, k * 128:(k + 1) * 128],
                            rhs=gt[0:cin, :], start=True, stop=True)
                        sy = epool.tile([P, gw], f32, name=f"sy{tag}_{k}_{b0}", tag="sy")
                        nc.vector.tensor_copy(out=sy[:, :], in_=sacc[:, :])
                        sot = otpool.tile([P, gw], f32, name=f"sot{tag}_{k}_{b0}",
                                          tag="ot", space="PSUM")
                        for j in range(nb):
                            nc.tensor.transpose(
                                out=sot[:, j * 128:(j + 1) * 128],
                                in_=sy[:, j * 128:(j + 1) * 128], identity=ident[:])
                        nc.scalar.copy(out=sc_sb[:, b0 * 128:b0 * 128 + gw], in_=sot[:, :])
                    nc.gpsimd.dma_scatter_add(
                        corr[tag][:, :],
                        sc_sb[:, :].rearrange("p (b c) -> p b c", c=128),
                        si[:, col * 8:(col + tks) * 8],
                        tks * 128, tks * 128, COUT,
                        queue_num=(ki % NQ_G),
                    )
                    col += tks

            def dense_phase(tag):
                cin = cin_t[tag]
                ft = ft_t[tag]
                cidx = {"00": 0, "01": 3, "10": 6, "11": 9}[tag]
                s_ap = bn_sb[:, cidx:cidx + 1]
                s2_ap = bn_sb[:, cidx + 1:cidx + 2]
                b_ap = bn_sb[:, cidx + 2:cidx + 3]
                for gidx in range(NG):
                    c0 = gidx * GRP
                    rhs = epool.tile([cin, GRP], f32, name=f"rhs{tag}_{gidx}", tag="rhs")
                    nc.sync.dma_start(out=rhs[:, :], in_=ft[:, c0:c0 + GRP])
                    crr_bf = epool.tile([P, 4, 128], mybir.dt.bfloat16,
                                        name=f"crrb{tag}_{gidx}", tag="crrb")
                    nc.sync.dma_start(
                        out=crr_bf[:, :, :],
                        in_=corr[tag][c0:c0 + GRP, :].rearrange("(j p) c -> p j c", p=128))
                    crr = epool.tile([P, 4, 128], f32, name=f"crr{tag}_{gidx}", tag="crr")
                    nc.vector.tensor_copy(out=crr[:, :, :], in_=crr_bf[:, :, :])
                    acc = accpool.tile([P, GRP], f32, name=f"dacc{tag}_{gidx}",
                                       tag="acc", space="PSUM")
                    nc.tensor.matmul(out=acc[:, :],
                                     lhsT=w_sb[tag][:, 4 * 128:5 * 128],
                                     rhs=rhs[:, :], start=True, stop=False)
                    for j in range(4):
                        nc.tensor.matmul(out=acc[:, j * 128:(j + 1) * 128],
                                         lhsT=crr[:, j, :], rhs=ident[:],
                                         is_transpose=True, start=False, stop=(j == 3),
                                         skip_group_check=True)
                    t1 = epool.tile([P, GRP], f32, name=f"t1{tag}_{gidx}", tag="t1")
                    nc.scalar.activation(t1[:, :], acc[:, :],
                                         mybir.ActivationFunctionType.Identity,
                                         bias=b_ap, scale=s_ap)
                    r2 = epool.tile([P, GRP], f32, name=f"r2{tag}_{gidx}", tag="r2")
                    nc.scalar.activation(r2[:, :], acc[:, :],
                                         mybir.ActivationFunctionType.Relu,
                                         bias=0.0, scale=s2_ap)
                    y = epool.tile([P, GRP], f32, name=f"y{tag}_{gidx}", tag="y")
                    nc.vector.tensor_add(out=y[:, :], in0=t1[:, :], in1=r2[:, :])
                    if tag == "00":
                        nc.sync.dma_start(out=ft_sc0[:, c0:c0 + GRP], in_=y[:, :])
                    elif tag == "10":
                        nc.sync.dma_start(out=ft_res0[:, c0:c0 + GRP], in_=y[:, :])
                    ot = otpool.tile([P, GRP], f32, name=f"dot{tag}_{gidx}",
                                     tag="ot", space="PSUM")
                    for j in range(4):
                        nc.tensor.transpose(out=ot[:, j * 128:(j + 1) * 128],
                                            in_=y[:, j * 128:(j + 1) * 128],
                                            identity=ident[:])
                    yr = epool.tile([P, GRP], f32, name=f"yr{tag}_{gidx}", tag="yr")
                    nc.vector.tensor_copy(out=yr[:, :], in_=ot[:, :])
                    rview = "p (j c) -> p j c"
                    if tag == "00":
                        nc.sync.dma_start(
                            out=sc0_row[c0:c0 + GRP, :].rearrange("(j p) c -> p j c", p=128),
                            in_=yr[:, :].rearrange(rview, c=128))
                    elif tag == "10":
                        nc.sync.dma_start(
                            out=res0_row[c0:c0 + GRP, :].rearrange("(j p) c -> p j c", p=128),
                            in_=yr[:, :].rearrange(rview, c=128))
                    elif tag == "01":
                        nc.sync.dma_start(
                            out=sc_d[c0:c0 + GRP, :].rearrange("(j p) c -> p j c", p=128),
                            in_=yr[:, :].rearrange(rview, c=128))
                    else:
                        sct = epool.tile([P, GRP], f32, name=f"sct{tag}_{gidx}", tag="sct")
                        nc.sync.dma_start(
                            out=sct[:, :].rearrange(rview, c=128),
                            in_=sc_d[c0:c0 + GRP, :].rearrange("(j p) c -> p j c", p=128))
                        yf = epool.tile([P, GRP], f32, name=f"yf{tag}_{gidx}", tag="yf")
                        nc.vector.tensor_add(out=yf[:, :], in0=yr[:, :], in1=sct[:, :])
                        nc.sync.dma_start(
                            out=out_d[c0:c0 + GRP, :].rearrange("(j p) c -> p j c", p=128),
                            in_=yf[:, :].rearrange(rview, c=128))

            sparse_phase("00")
            sparse_phase("10")
            dense_phase("00")
            if not timing:
                nc.gpsimd.collective_compute(
                    "AllGather", mybir.AluOpType.bypass,
                    replica_groups=[list(range(NCORES))],
                    ins=[sc0_row[:, :]], outs=[ag_sc0[:, :]])
            dense_phase("10")
            if not timing:
                nc.gpsimd.collective_compute(
                    "AllGather", mybir.AluOpType.bypass,
                    replica_groups=[list(range(NCORES))],
                    ins=[res0_row[:, :]], outs=[ag_res0[:, :]])
            sparse_phase("01")
            dense_phase("01")
            sparse_phase("11")
            dense_phase("11")

    nc.compile()
    return nc


NQ_G = 2
FSROWS = [0]  # static compacted sparse-source row count (set before build)

_prog_cache = {}
_last_skels = [None]  # for test-side timing builds


def _skel_key(skels):
    return tuple((tag, tuple((k, tuple(w)) for (k, w) in skels[tag]))
                 for tag in ("00", "10", "01", "11"))


def kernel(feats, W00, W01, W10, W11, bn00, bn01, bn10, bn11, nbr133, nbr313):
    feats = np.ascontiguousarray(np.asarray(feats, np.float32))
    nbr133 = np.asarray(nbr133, np.int32)
    nbr313 = np.asarray(nbr313, np.int32)

    # The kernel folds the k=4 (0,0,0) offset into a dense identity matmul.
    ar = np.arange(N, dtype=np.int64)
    if not (np.array_equal(nbr133[4], ar) and np.array_equal(nbr313[4], ar)):
        raise NotImplementedError("k=4 is not the identity offset")

    # Per-core compacted gather source for the feats-based convs: only the
    # rows actually referenced by the core's valid non-center entries.
    rows_c = []
    for c in range(NCORES):
        js = []
        for nbr in (nbr133, nbr313):
            seg = nbr[KS, c * SHARD:(c + 1) * SHARD]
            js.append(seg[seg >= 0].astype(np.int64))
        rows_c.append(np.unique(np.concatenate(js)))
    fsmax = max(1, max(len(r) for r in rows_c))
    fsmax = -(-fsmax // 128) * 128
    FSROWS[0] = fsmax

    feat_pos = lambda c, j: np.searchsorted(rows_c[c], j)
    shard_pos = lambda c, j: (j // SHARD) * SLOT + j % SHARD

    skel00, g00, s00 = _sparse_plan(nbr133, feat_pos, fsmax)
    skel10, g10, s10 = _sparse_plan(nbr313, feat_pos, fsmax)
    skel01, g01, s01 = _sparse_plan(nbr313, shard_pos, AGR)
    skel11, s11g, s11s = _sparse_plan(nbr133, shard_pos, AGR)
    skels = {"00": skel00, "10": skel10, "01": skel01, "11": skel11}
    g11, s11 = s11g, s11s
    _last_skels[0] = skels

    key = (_skel_key(skels), fsmax)
    nc = _prog_cache.get(key)
    if nc is None:
        nc = build_program(skels)
        _prog_cache.clear()
        _prog_cache[key] = nc

    # weights: [9, ci, co] -> [ci, 9*co]
    wpack = lambda W: np.ascontiguousarray(
        np.asarray(W, np.float32).transpose(1, 0, 2).reshape(np.asarray(W).shape[1], -1))
    w00p, w10p = wpack(W00), wpack(W10)
    w01p, w11p = wpack(W01), wpack(W11)
    bnv = np.zeros((P, 12), np.float32)
    for i, bn in enumerate((bn00, bn01, bn10, bn11)):
        s, s2, b = _bn_vecs(bn)
        col = {0: 0, 1: 3, 2: 6, 3: 9}[i]
        bnv[:, col], bnv[:, col + 1], bnv[:, col + 2] = s, s2, b

    in_maps = []
    for c in range(NCORES):
        ftf = np.zeros((CIN, SLOT), np.float32)
        ftf[:, :SHARD] = feats[c * SHARD:(c + 1) * SHARD].T
        fsrc = np.zeros((fsmax, CIN), np.float32)
        fsrc[:len(rows_c[c])] = feats[rows_c[c]]
        in_maps.append({
            "feats_row": fsrc,
            "ftf": ftf,
            "w00": w00p, "w10": w10p, "w01": w01p, "w11": w11p,
            "bnv": bnv,
            "gidx00": g00[c], "sidx00": s00[c],
            "gidx10": g10[c], "sidx10": s10[c],
            "gidx01": g01[c], "sidx01": s01[c],
            "gidx11": g11[c], "sidx11": s11[c],
        })

    res = run_bass_kernel_spmd(nc, in_maps, core_ids=list(range(NCORES)))
    out = np.empty((N, COUT), np.float32)
    for c in range(NCORES):
        out[c * SHARD:(c + 1) * SHARD] = res.results[c]["out"][:SHARD]
    return out
